# revision 18
# baseline (speedup 1.0000x reference)
"""Trainium2 Bass kernel for nn_GN_89266600280080.

Computes, for output[B,O], input[B,D], weights[O]:
    dl_dW = (1/B) * (output * weights)^T @ input        # [O, D]
    gw    = sqrt(sum(dl_dW^2, axis=1))                  # [O]

Strategy (8 NeuronCores, data-parallel over batch):
  - shard output/input on dim 0 across the 8 cores (B_loc = B/8 = 4096)
  - per core: M_partial = output_loc^T @ input_loc accumulated in PSUM
    via 128-deep K tiles on the tensor engine, float32r (1 cycle/row
    for moving free >= 256, vs 4 for plain fp32; weights fold deferred
    to the final [O]-sized sqrt)
  - ReduceScatter(add) the [O, D] partial: core c receives the true
    row-block o in [4c, 4c+4) x D  (cheaper than AllReduce: no 1.875x
    collective penalty, and the squared-norm reduction that follows is
    local)
  - per core: ssp[r] = sum_d rs[r, d]^2  (DVE square + reduce), DMA'd
    out as this core's [4]-element result
  - host gathers the 8 x [4] partials (kernel() already receives every
    core's outputs) and finishes with gw = sqrt(ss) * |w| / B on 32
    floats -- no AllGather, no on-device sqrt, no weights upload
"""

import sys
import numpy as np

for _p in ("/opt/trn_rl_repo", "/root/.axon_site/_ro/trn_rl_repo"):
    if _p not in sys.path:
        sys.path.insert(0, _p)

B, O, D = 32768, 32, 1024
N_CORES = 8
B_LOC = B // N_CORES
P = 128                 # partitions per K tile
NMM = 512               # moving-operand free dim per matmul
O_SC = O // N_CORES     # rows of dl_dW each core owns after ReduceScatter


def build(b_loc=B_LOC, n_cores=N_CORES, b_total=B, ch=4, n_iters=1):
    """Build + compile the per-core Bass program. Returns the Bacc object."""
    import concourse.bacc as bacc
    import concourse.tile as tile
    import concourse.mybir as mybir

    f32 = mybir.dt.float32
    f32r = mybir.dt.float32r
    kt = b_loc // P          # K tiles per core
    assert b_loc % P == 0 and kt % ch == 0
    nh = D // NMM

    nc = bacc.Bacc("TRN2", target_bir_lowering=False, debug=False,
                   num_devices=n_cores)

    out_d = nc.dram_tensor("output", [b_loc, O], f32r, kind="ExternalInput")
    in_d = nc.dram_tensor("input", [b_loc, D], f32r, kind="ExternalInput")
    ssp_d = nc.dram_tensor("ssp", [O // n_cores], f32, kind="ExternalOutput")

    out_ap = out_d.ap().rearrange("(n p) o -> p n o", p=P)
    in_ap = in_d.ap().rearrange("(n p) d -> p n d", p=P)

    with tile.TileContext(nc) as tc:
        with (
            tc.tile_pool(name="wout", bufs=2) as wout_pool,
            tc.tile_pool(name="rhs", bufs=3) as rhs_pool,
            tc.tile_pool(name="ps", bufs=2, space="PSUM") as psum_pool,
            tc.tile_pool(name="misc", bufs=2) as misc,
            tc.tile_pool(name="dram", bufs=2, space="DRAM") as dram_pool,
        ):
            for _it in range(n_iters):
                # stationary operand: all local w_out rows, [128, kt, O]
                wout = wout_pool.tile([P, kt, O], f32r)
                nc.sync.dma_start(wout[:], out_ap)

                # chunk sizes taper at the end so the final DMA->matmul
                # tail on the critical path is short
                sizes = [ch] * 6 + [2, 2, 2, 1, 1]
                assert sum(sizes) == kt
                psum = psum_pool.tile([O, D], f32)
                k = 0
                for ci, cs in enumerate(sizes):
                    rhs = rhs_pool.tile([P, cs, D], f32r, name=f"rhs{ci % 3}")
                    nc.scalar.dma_start(
                        rhs[:], in_ap[:, k:k + cs, :])
                    for j in range(cs):
                        for h in range(nh):
                            nc.tensor.matmul(
                                psum[:, h * NMM:(h + 1) * NMM],
                                wout[:, k, :],
                                rhs[:, j, h * NMM:(h + 1) * NMM],
                                start=(k == 0),
                                stop=(k == kt - 1),
                            )
                        k += 1

                # PSUM -> SBUF -> DRAM (DMA cannot read PSUM).  NOTE: a
                # K-split two-phase ReduceScatter pipeline was tried and
                # does NOT help: collectives drain all outstanding DMAs
                # first, so a mid-stream RS still waits for the full input.
                part_sb = misc.tile([O, D], f32)
                nc.vector.tensor_copy(part_sb[:], psum[:])
                part_dram = dram_pool.tile([O, D], f32)
                nc.sync.dma_start(part_dram[:], part_sb[:])

                # ReduceScatter(add): core at group index c receives the
                # summed rows [O_SC*c, O_SC*(c+1)) x D of dl_dW (x B)
                rs_dram = dram_pool.tile([O_SC, D], f32)
                nc.gpsimd.collective_compute(
                    "ReduceScatter",
                    mybir.AluOpType.add,
                    replica_groups=[list(range(n_cores))],
                    ins=[part_dram.opt()],
                    outs=[rs_dram.opt()],
                )

                rs_sb = misc.tile([O_SC, D], f32)
                nc.sync.dma_start(rs_sb[:], rs_dram[:])

                # ssp[r] = sum_d rs[r,d]^2 in ONE scalar-engine op:
                # Square activation with free-axis accumulate.  (The DVE
                # tensor_tensor_reduce alternative crashes the runtime:
                # NRT_EXEC_UNIT_UNRECOVERABLE.)
                sq = misc.tile([O_SC, D], f32)
                ssp = misc.tile([O_SC, 1], f32)
                nc.scalar.activation(
                    sq[:], rs_sb[:], mybir.ActivationFunctionType.Square,
                    accum_out=ssp[:])

                nc.sync.dma_start(
                    ssp_d.ap().rearrange("(p one) -> p one", one=1),
                    ssp[:])

    nc.compile()
    return nc


_CACHE = {}


def _get_nc():
    if "nc" not in _CACHE:
        _CACHE["nc"] = build()
    return _CACHE["nc"]


def kernel(output, input, weights):
    from concourse.bass_utils import run_bass_kernel_spmd

    output = np.asarray(output, dtype=np.float32)
    input = np.asarray(input, dtype=np.float32)
    weights = np.asarray(weights, dtype=np.float32)

    nc = _get_nc()
    in_maps = [
        {
            "output": output[c * B_LOC:(c + 1) * B_LOC],
            "input": input[c * B_LOC:(c + 1) * B_LOC],
        }
        for c in range(N_CORES)
    ]
    res = run_bass_kernel_spmd(nc, in_maps, list(range(N_CORES)))
    ss = np.concatenate(
        [np.asarray(res.results[c]["ssp"], dtype=np.float32).reshape(O_SC)
         for c in range(N_CORES)]
    )
    return (np.sqrt(ss) * np.abs(weights) / B).astype(np.float32)
